# revision 3
# baseline (speedup 1.0000x reference)
"""Single-head causal attention (B=8, T=2048, C=256, H=64) on 8 TRN2 NeuronCores.

Sharding: batch dim across the 8 cores (data parallel, one batch element per
core); each core computes its full TxT causal attention independently.

v2 — restructured from the 47.7us baseline around the sim timeline:
  * prologue pipelined per 512-row tq block: one packed-weight DMA, then
    DMA xt(j) -> proj(j) -> QK(j) while xt(j+1) streams.
  * proj_b ([kT;qT], the row-swapped copy dual-row-group QK needs) comes from
    a second packed weight matmul (wb = [Wk|Wq]) instead of SBUF->SBUF DMAs.
  * v is produced directly in PV-weight layout ([t,h] chunks) by using xt
    chunks as the stationary operand -- no PE transposes for v.
  * epilogue transposes are normal matmuls against a 65x65 identity (cheap,
    HAM-warm) instead of transpose-mode ops.
  * a dummy exp at t=0 hoists the ~2.4us ACT table load off the critical path.
  * diagonal pairs run first within each block so the tail pair needs no
    mask; the last store is split across the SP and ACT DMA rings.

Per-core algorithm (x_b = x[b], [T, C]):
  proj_a = [Wq|Wk].T @ x_b.T   [128, T] rows 0:64 = qT, 64:128 = kT
  proj_b = [Wk|Wq].T @ x_b.T   [128, T] rows 0:64 = kT, 64:128 = qT
  vaug[t,:] = [x_b @ Wv ; 1]   [T, 65]  (PV stationary operand)
  weiT tiles via dual row-group QK (even chunk rows 0-63, odd rows 64-127)
  e = exp(0.125 * weiT), causal: diagonal 128-tiles masked by tri, fully
      masked tile pairs never computed
  po[h, tq] = sum_i vaug_i.T @ e_i  (row 64 = softmax denominator)
  out = (po[:64] / po[64]).T  via identity-matmul transpose + reciprocal
"""

import os

import numpy as np

import concourse.bass as bass
import concourse.mybir as mybir
import concourse.tile as tile
from concourse import bass_utils

B, T, C, H = 8, 2048, 256, 64
NCC = C // 128          # 2 c-chunks
NTQ = T // 512          # 4 tq blocks
NTK = T // 128          # 16 tk chunks
WW = 128 + 128 + H      # packed [wa|wb|wv] free width

MASK_ENG = os.environ.get("V2_MASK_ENG", "gpsimd")   # gpsimd | dve | none
SKIP = set(os.environ.get("V2_SKIP", "").split(","))   # {exp, pv, epi} timing probes
SCH = int(os.environ.get("V2_SCH", "0"))   # exp pairs offloaded to DVE
# DVE bf16 Schraudolph exp: bits16 = round(w*SCH_A + SCH_B) viewed as bf16
# approximates exp(0.125*w) to +-3.3% rel; offloaded pairs relieve the
# saturated ACT engine. Constants fit empirically (C=5.6 minimax).
SCH_A = 23.083120654223414      # 0.125 * 128 * log2(e)
SCH_B = 128.0 * 127 - 5.6

dt = mybir.dt
# matmul-operand dtype: bf16 (default) gets FWL weight loads, 2x DVE copy
# modes and half the DMA/SBUF traffic; fp32r is the full-precision fallback.
MM_DT = dt.bfloat16 if os.environ.get("V2_DT", "bf16") == "bf16" else dt.float32r
F32 = dt.float32


def _split_excess_waits(nc, max_waits=1):
    """The walrus build in this container rejects >1 sync wait per
    instruction ("Too many sync wait commands"); spill extras onto
    preceding same-engine NoOps (same AND semantics, engine blocks at the
    NoOp until the semaphore condition holds)."""
    for f in nc.m.functions:
        for bb in f.blocks:
            new = []
            for inst in bb.instructions:
                si = inst.sync_info
                waits = list(si.on_wait) if si is not None else []
                if len(waits) > max_waits:
                    extra, keep = waits[:-max_waits], waits[-max_waits:]
                    for i in range(0, len(extra), max_waits):
                        chunk = extra[i:i + max_waits]
                        nop = mybir.InstNoOp(
                            name=nc.get_next_instruction_name(),
                            engine=inst.engine,
                            ins=[], outs=[],
                            sync_info=mybir.SyncInfo(on_wait=chunk, on_update=[]),
                        )
                        nc.register_instruction(nop)
                        new.append(nop)
                    inst.sync_info = mybir.SyncInfo(
                        on_wait=keep, on_update=list(si.on_update))
                new.append(inst)
            bb.instructions = new


def _patch_tile_drain():
    """Tile's kernel-tail drain carries one wait per live semaphore; split
    them the same way (idempotent monkeypatch)."""
    from concourse.vector_clock import ScopedClock

    if getattr(tile.TileContext, "_ant_drain_patched", False):
        return

    def _drain_and_barrier(self, tick_clock, wait_clock):
        drain_inst = self.nc.sync.drain()
        wait_clock.add_sem_waits(
            drain_inst.ins, ScopedClock({None: tick_clock.global_clock}))
        si = drain_inst.ins.sync_info
        waits = list(si.on_wait) if si is not None else []
        if len(waits) > 1:
            drain_inst.ins.sync_info = mybir.SyncInfo(
                on_wait=[waits[0]], on_update=list(si.on_update))
            for w in waits[1:]:
                ni = self.nc.sync.nop(nofuse=True)
                ni.ins.sync_info = mybir.SyncInfo(on_wait=[w], on_update=[])
        self.nc.all_engine_barrier()
        assert self.sems is not None
        popped = self.nc._tile_sem_poison_stack.pop()
        assert popped is self._sem_poison
        self.nc.clear_and_free_semaphores(list(self.sems.allocated().values()))
        self.nc.all_engine_barrier()

    tile.TileContext._drain_and_barrier = _drain_and_barrier
    tile.TileContext._ant_drain_patched = True


def _attention_body(nc, tc, pools, dram):
    """Emit one pass of the per-core attention computation."""
    persist, epool, onat, spool, psw, pso, pst = pools
    # mid-block non-diagonal pairs eligible for the DVE exp path
    sch_pairs = [(3, 3), (3, 5), (2, 3), (1, 2)][:SCH]
    xt_d, ww_d, tri_d, idn_d, ones_d, out_d = dram
    Exp = mybir.ActivationFunctionType.Exp
    Copy = mybir.ActivationFunctionType.Copy

    # ---- persistent SBUF tensors -------------------------------------
    xt = persist.tile([128, NCC, T], MM_DT, tag="xt")
    ww = persist.tile([128, NCC, WW], MM_DT, tag="ww")
    wa = ww[:, :, 0:128]
    wb = ww[:, :, 128:256]
    wv = ww[:, :, 256:WW]
    tri = persist.tile([128, 128], MM_DT, tag="tri")
    idn = persist.tile([128, 128], F32, tag="idn")
    proj_a = persist.tile([128, T], MM_DT, tag="proj_a")   # [qT; kT]
    proj_b = persist.tile([128, T], MM_DT, tag="proj_b")   # [kT; qT]
    vaug = persist.tile([128, NTK, H + 1], MM_DT, tag="vaug")
    dmy = persist.tile([1, 2], F32, tag="dmy")

    # ---- t=0: ACT table warm-up + ones column + weight/x DMAs --------
    nc.gpsimd.memset(dmy[:], 0.0)
    nc.scalar.activation(dmy[0:1, 1:2], dmy[0:1, 0:1], Exp)
    nc.sync.dma_start(vaug[:, :, H:H + 1], ones_d[:])

    nc.sync.dma_start(ww[:], ww_d.rearrange("c p k -> p c k"))
    # HAM warm-up: the For_i barrier leaves the PE throttled to 1.2 GHz at
    # every iteration start; a burst of tiny matmuls during the xt DMA wait
    # trips the activity monitor so the real prologue runs at 2.4 GHz.
    wrm = psw.tile([128, 1024], F32, tag="w")
    for r in range(14):
        nc.tensor.matmul(wrm[0:64, 0:64], ww[:, 0, 0:64], ww[:, 0, 0:64],
                         start=(r == 0), stop=(r == 13), skip_group_check=True)
    xt_loaded = [False] * NTQ

    def dma_xt(j):
        if not xt_loaded[j]:
            for cc in range(NCC):
                nc.sync.dma_start(xt[:, cc, 512 * j:512 * (j + 1)],
                                  xt_d[cc, :, 512 * j:512 * (j + 1)])
            xt_loaded[j] = True

    dma_xt(3)
    nc.sync.dma_start(tri[:], tri_d[:])
    nc.sync.dma_start(idn[:], idn_d[:])

    def prologue_qk(j, first=False):
        """proj_a/proj_b for tq block j (xt(j) must be DMA-queued)."""
        sl = slice(512 * j, 512 * (j + 1))
        pp = psw.tile([128, 1024], F32, tag="w")
        for cc in range(NCC):
            nc.tensor.matmul(pp[:, 0:512], wa[:, cc, :], xt[:, cc, sl],
                             start=(cc == 0), stop=(cc == NCC - 1))
        for cc in range(NCC):
            nc.tensor.matmul(pp[:, 512:1024], wb[:, cc, :], xt[:, cc, sl],
                             start=(cc == 0), stop=(cc == NCC - 1))
        nc.vector.tensor_copy(proj_a[:, sl], pp[:, 0:512])
        if first:
            # ACT is idle pre-loop; take the second copy off DVE's queue
            nc.scalar.activation(proj_b[:, sl], pp[:, 512:1024], Copy)
        else:
            nc.vector.tensor_copy(proj_b[:, sl], pp[:, 512:1024])

    def prologue_v(j):
        """v for block j in natural [t, h] layout: xt chunks stationary."""
        pv = pst.tile([128, 4, H + 2], F32, tag="t")
        for t in range(4):
            i = 4 * j + t
            for cc in range(NCC):
                nc.tensor.matmul(pv[:, t, 0:H], xt[:, cc, 128 * i:128 * (i + 1)],
                                 wv[:, cc, :],
                                 start=(cc == 0), stop=(cc == NCC - 1))
        nc.vector.tensor_copy(vaug[:, 4 * j:4 * j + 4, 0:H], pv[:, :, 0:H])

    prologue_qk(3, first=True)
    prologue_v(3)
    dma_xt(0)
    dma_xt(1)
    dma_xt(2)

    # ---- main loop over tq blocks ------------------------------------
    def masks(epack):
        # on GPSIMD: keeps DVE free for the prologue PSUM->SBUF copies;
        # diagonal pairs run early in each block so their PV has slack
        e, i0, i1, c0, c1, mlist = epack
        if MASK_ENG == "none":
            return
        eng = nc.gpsimd if MASK_ENG == "gpsimd" else nc.vector
        for off, d in mlist:
            eng.tensor_mul(
                e[:, off + 128 * d:off + 128 * (d + 1)],
                e[:, off + 128 * d:off + 128 * (d + 1)], tri[:])

    def epi2(pend, last=False):
        """Deferred epilogue: transpose ot via identity matmuls, normalize,
        store. Emitted inside the NEXT block so the PE queue never stalls
        on the ot copy."""
        if "epi" in SKIP:
            return
        jj, ot = pend
        pt = pst.tile([128, 4, H + 2], F32, tag="t")
        for t in range(4):
            nc.tensor.matmul(pt[:, t, :], ot[:, 128 * t:128 * (t + 1)],
                             idn[0:H + 1, 0:H + 2], start=True, stop=True)
        rc = onat.tile([128, 4, 1], F32, tag="rc")
        nc.vector.reciprocal(rc[:], pt[:, :, H:H + 1])
        on = onat.tile([128, 4, H], F32, tag="on")
        nc.vector.tensor_mul(on[:], pt[:, :, 0:H],
                             rc[:].to_broadcast([128, 4, H]))
        dst = out_d[512 * jj:512 * (jj + 1)].rearrange("(t p) h -> p t h",
                                                       p=128)
        if last:
            # tail: split the last store across both HWDGE rings (ACT's
            # queue is empty by now)
            nc.sync.dma_start(dst[:, 0:2, :], on[:, 0:2, :])
            nc.scalar.dma_start(dst[:, 2:4, :], on[:, 2:4, :])
        else:
            nc.sync.dma_start(dst, on[:])

    # Block order [3, 2, 0, 1]: the long block 3 runs first and all other
    # prologues complete inside its ~8us of ACT runway, so the later block
    # transitions carry no projection dependency (the per-block DVE copy
    # chain was the main ACT-gap source). The last block (1) ends on a
    # non-diagonal pair, keeping masks off the tail chain.
    prolog_sched = {(3, 0): [("qk", 0)], (3, 1): [("qk", 1), ("v", 0)],
                    (3, 2): [("qk", 2), ("v", 1)], (3, 4): [("v", 2)]}
    pending = None
    for j in (3, 2, 0, 1):
        nk = 4 * j + 4                      # valid tk chunks (causal)
        po = pso.tile([H + 1, 512], F32, tag="o")
        # diagonal pairs first: tail pair of each block then needs no mask
        # before its PV, and diagonal masks get mid-block slack
        order = [2 * j, 2 * j + 1] + list(range(2 * j))
        es = []
        first_pv = True
        for pi, p in enumerate(order):
            i0, i1 = 2 * p, 2 * p + 1
            wp = psw.tile([128, 1024], F32, tag="w")
            # dual row-group QK: even chunk on PE rows 0-63, odd on 64-127.
            # Diagonal chunks (d >= 0) skip their fully-masked left columns.
            c0 = 128 * max(0, i0 - 4 * j)
            c1 = 128 * max(0, i1 - 4 * j)
            nc.tensor.matmul(wp[:, c0:512],
                             proj_b[0:64, 128 * i0:128 * (i0 + 1)],
                             proj_a[0:64, 512 * j + c0:512 * (j + 1)],
                             start=True, stop=True)
            nc.tensor.matmul(wp[:, 512 + c1:1024],
                             proj_a[64:128, 128 * i1:128 * (i1 + 1)],
                             proj_b[64:128, 512 * j + c1:512 * (j + 1)],
                             start=True, stop=True, tile_position=(64, 0))
            e = epool.tile([128, 1024], MM_DT, tag="e")
            d0, d1 = i0 - 4 * j, i1 - 4 * j
            mlist = [] if d1 < 0 else [(0, d0), (512, d1)]
            if "exp" in SKIP:
                mlist = []
            elif d1 < 0 and (j, pi) in sch_pairs:
                y = spool.tile([128, 1024], F32, tag="y")
                nc.vector.tensor_scalar(y[:], wp[:], SCH_A, SCH_B,
                                        op0=mybir.AluOpType.mult,
                                        op1=mybir.AluOpType.add)
                nc.vector.tensor_copy(e[:].bitcast(dt.int16), y[:])
            elif d1 < 0:        # both tiles fully unmasked
                nc.scalar.activation(e[:], wp[:], Exp, scale=0.125)
            elif j == 3 and pi == 0:
                # shortest critical path into the loop: exp the first QK
                # half as soon as matmul A lands
                nc.scalar.activation(e[:, 0:512], wp[:, 0:512], Exp,
                                     scale=0.125)
                nc.scalar.activation(e[:, 512 + 128:1024],
                                     wp[:, 512 + 128:1024], Exp, scale=0.125)
            else:
                # one ACT op spanning both halves (exp of the masked gap is
                # harmless garbage; neither masked nor read by PV)
                nc.scalar.activation(e[:, 128 * d0:1024],
                                     wp[:, 128 * d0:1024], Exp, scale=0.125)
            es.append((e, i0, i1, c0, c1, mlist))
            for kind, pj in prolog_sched.get((j, pi), ()):
                # PE/DVE fill while ACT chews this block
                (prologue_qk if kind == "qk" else prologue_v)(pj)
            if pi == 2 and pending is not None:
                epi2(pending)
                pending = None
            # software-pipeline PV one pair behind QK (masks emitted just
            # before their PV so prologue copies get DVE priority)
            if len(es) >= 2:
                masks(es[-2])
                _pv_pair(nc, po, vaug, es[-2], first_pv, False)
                first_pv = False
        masks(es[-1])
        _pv_pair(nc, po, vaug, es[-1], first_pv, True)
        if pending is not None:     # j0 -> j1: j1's pi==2 handled it already
            epi2(pending)
            pending = None

        # epilogue part 1: PSUM -> SBUF copy of the accumulated block
        ot = onat.tile([H + 1, 512], F32, tag="ot")
        if "epi" not in SKIP:
            nc.vector.tensor_copy(ot[:], po[:])
        pending = (j, ot)
    epi2(pending, last=True)


def _pv_pair(nc, po, vaug, epack, first, last):
    if "pv" in SKIP:
        return
    e, i0, i1, c0, c1, _ = epack
    nc.tensor.matmul(po[:, c0:512], vaug[:, i0, :], e[:, c0:512],
                     start=first, stop=False,
                     skip_group_check=True)
    nc.tensor.matmul(po[:, c1:512], vaug[:, i1, :],
                     e[:, 512 + c1:1024],
                     start=False, stop=last,
                     skip_group_check=True)


def build_nc(repeats=1, mm_dt=None):
    """Build the per-core Bass program (SPMD: same program on all 8 cores).

    repeats > 1 wraps the body in an on-device For_i loop; used only by the
    benchmarking harness to amortize host/launch overhead out of timing.
    """
    global MM_DT
    if mm_dt is not None:
        MM_DT = mm_dt
    _patch_tile_drain()
    nc = bass.Bass("TRN2", target_bir_lowering=False, debug=False)

    xt_d = nc.dram_tensor("xt", [NCC, 128, T], MM_DT, kind="ExternalInput")
    ww_d = nc.dram_tensor("ww", [NCC, 128, WW], MM_DT, kind="ExternalInput")
    tri_d = nc.dram_tensor("tri", [128, 128], MM_DT, kind="ExternalInput")
    idn_d = nc.dram_tensor("idn", [128, 128], F32, kind="ExternalInput")
    ones_d = nc.dram_tensor("ones", [128, NTK, 1], MM_DT, kind="ExternalInput")
    out_d = nc.dram_tensor("out", [T, H], F32, kind="ExternalOutput")
    dram = (xt_d, ww_d, tri_d, idn_d, ones_d, out_d)

    with tile.TileContext(nc) as tc:
        with (
            tc.tile_pool(name="persist", bufs=1) as persist,
            tc.tile_pool(name="epool", bufs=4) as epool,
            tc.tile_pool(name="onat", bufs=2) as onat,
            tc.tile_pool(name="spool", bufs=2) as spool,
            tc.tile_pool(name="psw", bufs=3, space="PSUM") as psw,
            tc.tile_pool(name="pso", bufs=1, space="PSUM") as pso,
            tc.tile_pool(name="pst", bufs=1, space="PSUM") as pst,
        ):
            pools = (persist, epool, onat, spool, psw, pso, pst)
            if repeats == 1:
                _attention_body(nc, tc, pools, dram)
            else:
                with tc.For_i(0, repeats, 1):
                    _attention_body(nc, tc, pools, dram)
    _split_excess_waits(nc)
    return nc


def make_in_maps(x, Wk, Wq, Wv):
    """Host-side layout prep: per-core transposed x, packed weights, masks."""
    x = np.asarray(x, dtype=np.float32)
    Wk = np.asarray(Wk, dtype=np.float32)
    Wq = np.asarray(Wq, dtype=np.float32)
    Wv = np.asarray(Wv, dtype=np.float32)

    md = mybir.dt.np(MM_DT)
    ww = np.concatenate([Wq, Wk, Wk, Wq, Wv], axis=1)  # [C, 320]
    ww = ww.reshape(NCC, 128, WW).astype(md)
    r = np.arange(128)
    tri = (r[:, None] <= r[None, :]).astype(md)  # keep tk <= tq
    idn = np.eye(128, dtype=np.float32)
    ones = np.ones((128, NTK, 1), dtype=md)
    common = {"ww": np.ascontiguousarray(ww), "tri": tri, "idn": idn,
              "ones": ones}
    in_maps = []
    for b in range(B):
        xt = np.ascontiguousarray(x[b].T.astype(md)).reshape(NCC, 128, T)
        in_maps.append({"xt": xt, **common})
    return in_maps


def kernel(x, Wk, Wq, Wv):
    nc = build_nc(repeats=1)
    in_maps = make_in_maps(x, Wk, Wq, Wv)
    res = bass_utils.run_bass_kernel_spmd(nc, in_maps, core_ids=list(range(B)))
    return np.stack([res.results[b]["out"] for b in range(B)], axis=0)
